# revision 24
# baseline (speedup 1.0000x reference)
"""Bilateral filter (3x3, sigma=0.8) Trainium2 Bass kernel.

Sharding: fully data-parallel over the fused batch B*V = 8 -> one
(C=3,H=512,W=512) image per NeuronCore, 8 cores.

Per-core layout: H=512 rows split 4 rows/partition over 128 partitions.
Each partition holds 6 rows (1 halo row above, 4 data rows, 1 halo row
below) x 520 cols (2 left pad, 512 data, 6 right pad) flattened in the
free dimension, so every 3x3 tap is a constant flat offset.

Math (validated vs reference, rel err ~2e-3 in fp16):
  out = num / den   (the 1e-7 eps term is dropped; |effect| ~ 1e-4)
  den = sum_k ws_k wd_k wc_k m[+k],  num_c = sum_k ws_k wd_k wc_k (m c_c)[+k]
with the tap pair symmetry: for e in {(0,1),(1,0),(1,1),(1,-1)}
  g_e(q)  = exp(-S (d(q+e)-d(q))^2)          (covers taps +e and -e)
  G_e(q)  = sum_c exp(-S (c(q+e)-c(q))^2)
  F_e     = ws_e * g_e * G_e
  den     = 3*ws0*m + sum_e [ (F_e*m[+e])@0 + (F_e*m)@-e ]
  num_c   = 3*ws0*m*c + sum_e [ (F_e*c[+e]*m[+e].. ) ... ]  via Ftm/Fhm fields
All tap sums accumulate on the TensorEngine (identity matmuls into PSUM).
"""

import math
import numpy as np
import sys

if "/opt/trn_rl_repo" not in sys.path:
    sys.path.insert(0, "/opt/trn_rl_repo")

import concourse.bass as bass
import concourse.tile as tile
from concourse import mybir
from concourse.bass_utils import run_bass_kernel_spmd

# ---- problem constants (hardcoded per spec) ----
B, V, C, H, W = 2, 4, 3, 512, 512
N_CORES = 8
KS = 3
SIG = 0.3 * ((KS - 1) * 0.5 - 1) + 0.8           # 0.8
S = 1.0 / (2.0 * SIG * SIG)                       # 0.78125

# spatial gaussian, normalized
_xs = np.arange(KS, dtype=np.float64)
_gx, _gy = np.meshgrid(_xs, _xs, indexing="xy")
_w = np.exp(-(((_gx - 1) ** 2 + (_gy - 1) ** 2)) * S)
_w = _w / _w.sum()
W0 = float(_w[1, 1])   # center
W1 = float(_w[0, 1])   # edge-adjacent
W2 = float(_w[0, 0])   # diagonal

# layout constants
R = 4                  # data rows per partition
W2C = 520              # row stride (2 left pad + 512 data + 6 right pad)
NROW = 6               # rows per partition incl. halo
FLAT = NROW * W2C      # 3120
ALLOC = FLAT + 16      # slack so reads at +521 from flat 2600 stay in-bounds
EXT = 5 * W2C          # 2600: field grid rows 0..4 (all cols)
PROD = 4 * W2C         # 2080: product-field length (4 rows)
OG = R * W            # 2048 output elems per partition
COL0 = 2               # first data col

# (er, ec, flat offset, spatial weight)
ES = [(0, 1, 1, W1), (1, 0, W2C, W1), (1, 1, W2C + 1, W2), (1, -1, W2C - 1, W2)]
IWI = [0, 0, 1, 1]          # which scaled identity each pair uses (w1 / w2)
SQS = math.sqrt(S)          # DErf(SQS*x) = 2/sqrt(pi) * exp(-S x^2)

F16 = mybir.dt.float16
F32 = mybir.dt.float32
AF = mybir.ActivationFunctionType
ALU = mybir.AluOpType


# ---- walrus single-wait workaround ----------------------------------------
# This container's walrus accepts only ONE sync_info.on_wait per instruction;
# Tile emits multi-wait instructions. Hoist all but the last wait onto
# injected single-wait instructions just before the original (NoOp for
# compute engines; a dummy 4-byte DMACopy on the same HWDGE queue for DMAs).
import orjson as _orjson

_SCRATCH = "wsplit_scratch"


def _mk_nop(name, engine, wait):
    return {"name": name, "engine": engine, "ins": [], "outs": [],
            "opcode": "NoOp",
            "sync_info": {"on_wait": [wait], "on_update": []}}


def _mk_dummy_dma(name, proto, wait):
    ap = {"ap": [[1, 1], [1, 1]], "dtype": "float32", "kind": "physical_ap",
          "memref": _SCRATCH, "memsetref": _SCRATCH + "_set", "offset": 0}
    d = {"name": name, "engine": proto["engine"], "opcode": "DMACopy",
         "mode": "Copy", "cce_op": "bypass", "single_packet": False,
         "ins": [ap], "outs": [dict(ap, offset=2)],
         "sync_info": {"on_wait": [wait], "on_update": []}}
    for k in ("queue", "oob_is_err"):
        if k in proto:
            d[k] = proto[k]
    return d


def _split_multiwaits(bir_bytes):
    m = _orjson.loads(bir_bytes)
    for f in m.get("functions", []):
        for bb in f.get("blocks", []):
            out = []
            for ins in bb.get("instructions", []):
                si = ins.get("sync_info")
                waits = (si or {}).get("on_wait") or []
                if len(waits) > 1:
                    for k, w in enumerate(waits[:-1]):
                        nm = f"{ins['name']}-wsplit{k}"
                        out.append(_mk_nop(nm, ins["engine"], w))
                    si["on_wait"] = [waits[-1]]
                out.append(ins)
            bb["instructions"] = out
    return _orjson.dumps(m)


_BUILD_CACHE = {}


def _build_nc():
    nc = bass.Bass()
    x_in = nc.declare_dram_parameter("x", [5, 128, NROW, W], F16, isOutput=False)
    id_in = nc.declare_dram_parameter("ident", [128, 128], F16, isOutput=False)
    idw_in = nc.declare_dram_parameter("identw", [2, 128, 128], F16, isOutput=False)
    o_out = nc.declare_dram_parameter("out", [C, H, W], F16, isOutput=True)
    nc.dram_tensor(_SCRATCH, [4], F32)

    with tile.TileContext(nc) as tc:
        _emit(nc, tc, x_in, id_in, idw_in, o_out)

    orig_to_json = nc.to_json_bytes
    nc.to_json_bytes = lambda: _split_multiwaits(orig_to_json())
    return nc


def _emit(nc, tc, x_in, id_in, idw_in, o_out):
    from contextlib import ExitStack
    ctx = ExitStack()
    with ctx:
        persist = ctx.enter_context(tc.tile_pool(name="persist", bufs=1))
        p1 = ctx.enter_context(tc.tile_pool(name="p1", bufs=1))
        p2 = ctx.enter_context(tc.tile_pool(name="p2", bufs=2))
        p3 = ctx.enter_context(tc.tile_pool(name="p3", bufs=3))
        yz_p = ctx.enter_context(tc.tile_pool(name="yz", bufs=2))
        psum_p = ctx.enter_context(
            tc.tile_pool(name="psum", bufs=1, space=bass.MemorySpace.PSUM)
        )

        # ---- persistent fp16 input planes ----
        d16 = persist.tile([128, ALLOC], F16, tag="d16", name="d16")
        m16 = persist.tile([128, ALLOC], F16, tag="m16", name="m16")
        c16all = persist.tile([128, C, ALLOC], F16, tag="c16all", name="c16all")
        c16 = [c16all[:, i, :] for i in range(C)]
        ident = persist.tile([128, 128], F16, tag="ident", name="ident")
        identw = persist.tile([128, 2, 128], F16, tag="identw", name="identw")
        m3w0 = persist.tile([128, R, W], F16, tag="m3w0", name="m3w0")
        fhm = [persist.tile([128, EXT], F16, tag=f"fhm{i}", name=f"fhm{i}") for i in range(4)]
        ftm = [persist.tile([128, PROD], F16, tag=f"ftm{i}", name=f"ftm{i}") for i in range(4)]
        r32 = persist.tile([128, R, W], F32, tag="r32", name="r32")

        # zero only the pad regions (halo rows/cols arrive zeroed from the
        # host): cols 0-1 and 514-519 of each of the 6 rows, plus the slack.
        def v3_(ap_flat):
            return ap_flat[:, 0:FLAT].rearrange("p (a b) -> p a b", b=W2C)

        for t in (d16, m16, *c16):
            nc.vector.memset(v3_(t[:])[:, :, 0:COL0], 0.0)
            nc.vector.memset(v3_(t[:])[:, :, COL0 + W:W2C], 0.0)
            nc.vector.memset(t[:, FLAT:ALLOC], 0.0)

        # ---- load the 5 fp16 halo'd planes (built host-side) ----
        # x_in[k] is [128, 6, 512]: per-partition rows 4p-1..4p+4 with zero
        # halos, already fp16. One contiguous DMA per plane, alternating
        # between the two HWDGE rings; pads in the on-chip 520-wide layout
        # are zeroed once by the gpsimd memsets above.
        def v3(ap_flat):
            return ap_flat[:, 0:FLAT].rearrange("p (a b) -> p a b", b=W2C)

        planes = [d16, c16[0], c16[1], c16[2], m16]  # c16[i]: AP slice
        for k, dst16 in enumerate(planes):
            eng = nc.sync if k % 2 == 0 else nc.scalar
            eng.dma_start(
                v3(dst16[:])[:, 0:6, COL0:COL0 + W], x_in[k]
            )
        nc.scalar.dma_start(ident[:], id_in[:])
        nc.scalar.dma_start(identw[:], idw_in.rearrange("j p c -> p j c"))

        def oview(t16):  # [128,4,512] output-grid view of a fp16 plane
            return v3(t16[:])[:, 1:5, COL0:COL0 + W]

        PHI2 = 4.0 / math.pi  # (2/sqrt(pi))^2 from the two DErf factors
        nc.vector.tensor_scalar_mul(m3w0[:], oview(m16), 3.0 * W0 * PHI2)


        den = psum_p.tile([128, R, W], F32, tag="acc", name="den", bufs=2)

        # ---- phase A: per-pair fields + den accumulation ----
        # ws_e folds into the depth exp bias (g' = exp(-S t^2 + ln ws_e));
        # er==0 pairs only need field rows 1..4 (range starts at W2C).
        for i, (er, ec, ef, wse) in enumerate(ES):
            lo = W2C if er == 0 else 0
            td = p3.tile([128, EXT], F16, tag="tt", name="td")
            g = p2.tile([128, EXT], F16, tag="g", name="g")
            nc.vector.tensor_sub(
                td[:, lo:EXT], d16[:, lo + ef:EXT + ef], d16[:, lo:EXT]
            )
            nc.scalar.activation(
                g[:, lo:EXT], td[:, lo:EXT], AF.Derivative_Erf, scale=SQS
            )

            # packed color chain: one [3 x n] op per stage; square+exp run
            # in place on the ACT engine
            tca = p3.tile([128, C, EXT], F16, tag="tca", name="tca", bufs=2)
            nc.vector.tensor_sub(
                tca[:, :, lo:EXT], c16all[:, :, lo + ef:EXT + ef],
                c16all[:, :, lo:EXT]
            )
            nc.scalar.activation(
                tca[:, :, lo:EXT], tca[:, :, lo:EXT], AF.Derivative_Erf,
                scale=SQS,
            )

            G = p1.tile([128, EXT], F16, tag="G", name="G")
            nc.vector.tensor_add(
                G[:, lo:EXT], tca[:, 0, lo:EXT], tca[:, 1, lo:EXT]
            )
            nc.vector.tensor_add(G[:, lo:EXT], G[:, lo:EXT], tca[:, 2, lo:EXT])
            F = p1.tile([128, EXT], F16, tag="F", name="F")
            nc.vector.tensor_mul(F[:, lo:EXT], g[:, lo:EXT], G[:, lo:EXT])
            fl, fh = (W2C, EXT) if er == 0 else (0, PROD)
            nc.vector.tensor_mul(
                fhm[i][:, fl:fh], F[:, fl:fh], m16[:, fl:fh]
            )
            nc.vector.tensor_mul(
                ftm[i][:], F[:, W2C:EXT], m16[:, W2C + ef:EXT + ef]
            )
            for r in range(R):
                o = r * W2C + COL0
                nc.tensor.matmul(
                    den[:, r, :], identw[:, IWI[i], :], ftm[i][:, o:o + W],
                    start=(i == 0), stop=False,
                )
                ob = (1 + r) * W2C + COL0 - ef
                nc.tensor.matmul(
                    den[:, r, :], identw[:, IWI[i], :], fhm[i][:, ob:ob + W],
                    start=False, stop=False,
                )

        for r in range(R):
            nc.tensor.matmul(
                den[:, r, :], ident[:], m3w0[:, r, :],
                start=False, stop=(r == R - 1),
            )

        # 1/den = exp(-ln(den)); den>0 always (products of exps, positive mask).
        # Ln+Exp share one ACT table set; custom-DVE recip ops don't compile
        # with this walrus build, and InstReciprocal is ~8 cyc/elem.
        lden = persist.tile([128, R, W], F32, tag="lden", name="lden")
        nc.scalar.activation(lden[:], den[:], AF.Ln)
        nc.scalar.activation(r32[:], lden[:], AF.Exp, scale=-1.0)

        # ---- phase B: per-channel numerators ----
        r16 = persist.tile([128, R, W], F16, tag="r16", name="r16")
        nc.vector.tensor_copy(r16[:], r32[:])

        def _finals(num, ci):
            # num (PSUM fp32) -> SBUF fp16 via ACT, then a 2x fp16 multiply;
            # the fp16 output is upcast on the host.
            n16 = p1.tile([128, R, W], F16, tag="n16", name="n16", bufs=2)
            nc.scalar.activation(n16[:], num[:], AF.Copy)
            o16 = p1.tile([128, R, W], F16, tag="o16", name="o16", bufs=2)
            nc.vector.tensor_mul(o16[:], n16[:], r16[:])
            (nc.sync if ci % 2 == 0 else nc.scalar).dma_start(
                o_out[ci].rearrange("(p r) w -> p r w", r=R), o16[:]
            )

        pending = None
        for ci in range(C):
            num = psum_p.tile([128, R, W], F32, tag="acc", name="num", bufs=2)
            ncc = p1.tile([128, R, W], F16, tag="ncc", name="ncc")
            nc.vector.tensor_mul(ncc[:], m3w0[:], oview(c16[ci]))
            for i, (er, ec, ef, wse) in enumerate(ES):
                fl, fh = (W2C, EXT) if er == 0 else (0, PROD)
                Y = yz_p.tile([128, PROD], F16, tag="Y", name="Y")
                Z = yz_p.tile([128, EXT], F16, tag="Z", name="Z")
                nc.vector.tensor_mul(
                    Y[:], ftm[i][:], c16[ci][:, W2C + ef:EXT + ef]
                )
                nc.vector.tensor_mul(
                    Z[:, fl:fh], fhm[i][:, fl:fh], c16[ci][:, fl:fh]
                )
                for r in range(R):
                    o = r * W2C + COL0
                    nc.tensor.matmul(
                        num[:, r, :], identw[:, IWI[i], :], Y[:, o:o + W],
                        start=(i == 0), stop=False,
                    )
                    ob = (1 + r) * W2C + COL0 - ef
                    nc.tensor.matmul(
                        num[:, r, :], identw[:, IWI[i], :], Z[:, ob:ob + W],
                        start=False, stop=False,
                    )
            for r in range(R):
                nc.tensor.matmul(
                    num[:, r, :], ident[:], ncc[:, r, :],
                    start=False, stop=(r == R - 1),
                )
            if pending is not None:
                _finals(*pending)
            pending = (num, ci)
        _finals(*pending)


def _get_nc():
    if "nc" not in _BUILD_CACHE:
        _BUILD_CACHE["nc"] = _build_nc()
    return _BUILD_CACHE["nc"]


def _halo_planes(d, c, m):
    """[N,5,128,6,512] fp16: per-partition 6-row windows with zero halos."""
    from numpy.lib.stride_tricks import as_strided
    planes = np.concatenate(
        [d[:, None], np.moveaxis(c, 1, 1), m[:, None]], axis=1
    ) if False else None
    stack = np.empty((N_CORES, 5, H + 2, W), np.float16)
    for i in range(N_CORES):
        for k, arr in enumerate((d[i], c[i, 0], c[i, 1], c[i, 2], m[i])):
            stack[i, k, 1:H + 1] = arr
    stack[:, :, 0] = 0.0
    stack[:, :, H + 1] = 0.0
    # window p = padded rows 4p .. 4p+5  (padded row r == image row r-1)
    s = stack.strides
    win = as_strided(
        stack,
        shape=(N_CORES, 5, 128, NROW, W),
        strides=(s[0], s[1], 4 * s[2], s[2], s[3]),
    )
    return np.ascontiguousarray(win)


def _run(depth, color, mask, trace=False, **kw):
    nc = _get_nc()
    d = np.asarray(depth, dtype=np.float32).reshape(N_CORES, H, W)
    c = np.asarray(color, dtype=np.float32).reshape(N_CORES, C, H, W)
    m = np.asarray(mask, dtype=np.float32).reshape(N_CORES, H, W)
    x16 = _halo_planes(d, c, m)
    ident = np.eye(128, dtype=np.float16)
    identw = np.stack([np.eye(128) * W1, np.eye(128) * W2]).astype(np.float16)
    in_maps = [
        {"x": x16[i], "ident": ident, "identw": identw} for i in range(N_CORES)
    ]
    res = run_bass_kernel_spmd(
        nc, in_maps, list(range(N_CORES)), trace=trace, **kw
    )
    out = np.stack([np.asarray(res.results[i]["out"]) for i in range(N_CORES)])
    return out.reshape(B, V, C, H, W).astype(np.float32), res


def kernel(depth, color, mask):
    out, _ = _run(depth, color, mask, trace=False)
    return out


# revision 25
# speedup vs baseline: 1.0032x; 1.0032x over previous
"""Bilateral filter (3x3, sigma=0.8) Trainium2 Bass kernel.

Sharding: fully data-parallel over the fused batch B*V = 8 -> one
(C=3,H=512,W=512) image per NeuronCore, 8 cores.

Per-core layout: H=512 rows split 4 rows/partition over 128 partitions.
Each partition holds 6 rows (1 halo row above, 4 data rows, 1 halo row
below) x 520 cols (2 left pad, 512 data, 6 right pad) flattened in the
free dimension, so every 3x3 tap is a constant flat offset.

Math (validated vs reference, rel err ~2e-3 in fp16):
  out = num / den   (the 1e-7 eps term is dropped; |effect| ~ 1e-4)
  den = sum_k ws_k wd_k wc_k m[+k],  num_c = sum_k ws_k wd_k wc_k (m c_c)[+k]
with the tap pair symmetry: for e in {(0,1),(1,0),(1,1),(1,-1)}
  g_e(q)  = exp(-S (d(q+e)-d(q))^2)          (covers taps +e and -e)
  G_e(q)  = sum_c exp(-S (c(q+e)-c(q))^2)
  F_e     = ws_e * g_e * G_e
  den     = 3*ws0*m + sum_e [ (F_e*m[+e])@0 + (F_e*m)@-e ]
  num_c   = 3*ws0*m*c + sum_e [ (F_e*c[+e]*m[+e].. ) ... ]  via Ftm/Fhm fields
All tap sums accumulate on the TensorEngine (identity matmuls into PSUM).
"""

import math
import numpy as np
import sys

if "/opt/trn_rl_repo" not in sys.path:
    sys.path.insert(0, "/opt/trn_rl_repo")

import concourse.bass as bass
import concourse.tile as tile
from concourse import mybir
from concourse.bass_utils import run_bass_kernel_spmd

# ---- problem constants (hardcoded per spec) ----
B, V, C, H, W = 2, 4, 3, 512, 512
N_CORES = 8
KS = 3
SIG = 0.3 * ((KS - 1) * 0.5 - 1) + 0.8           # 0.8
S = 1.0 / (2.0 * SIG * SIG)                       # 0.78125

# spatial gaussian, normalized
_xs = np.arange(KS, dtype=np.float64)
_gx, _gy = np.meshgrid(_xs, _xs, indexing="xy")
_w = np.exp(-(((_gx - 1) ** 2 + (_gy - 1) ** 2)) * S)
_w = _w / _w.sum()
W0 = float(_w[1, 1])   # center
W1 = float(_w[0, 1])   # edge-adjacent
W2 = float(_w[0, 0])   # diagonal

# layout constants
R = 4                  # data rows per partition
W2C = 520              # row stride (2 left pad + 512 data + 6 right pad)
NROW = 6               # rows per partition incl. halo
FLAT = NROW * W2C      # 3120
ALLOC = FLAT + 16      # slack so reads at +521 from flat 2600 stay in-bounds
EXT = 5 * W2C          # 2600: field grid rows 0..4 (all cols)
PROD = 4 * W2C         # 2080: product-field length (4 rows)
OG = R * W            # 2048 output elems per partition
COL0 = 2               # first data col

# (er, ec, flat offset, spatial weight)
ES = [(0, 1, 1, W1), (1, 0, W2C, W1), (1, 1, W2C + 1, W2), (1, -1, W2C - 1, W2)]
IWI = [0, 0, 1, 1]          # which scaled identity each pair uses (w1 / w2)
SQS = math.sqrt(S)          # DErf(SQS*x) = 2/sqrt(pi) * exp(-S x^2)

F16 = mybir.dt.float16
F32 = mybir.dt.float32
AF = mybir.ActivationFunctionType
ALU = mybir.AluOpType


# ---- walrus single-wait workaround ----------------------------------------
# This container's walrus accepts only ONE sync_info.on_wait per instruction;
# Tile emits multi-wait instructions. Hoist all but the last wait onto
# injected single-wait instructions just before the original (NoOp for
# compute engines; a dummy 4-byte DMACopy on the same HWDGE queue for DMAs).
import orjson as _orjson

_SCRATCH = "wsplit_scratch"


def _mk_nop(name, engine, wait):
    return {"name": name, "engine": engine, "ins": [], "outs": [],
            "opcode": "NoOp",
            "sync_info": {"on_wait": [wait], "on_update": []}}


def _mk_dummy_dma(name, proto, wait):
    ap = {"ap": [[1, 1], [1, 1]], "dtype": "float32", "kind": "physical_ap",
          "memref": _SCRATCH, "memsetref": _SCRATCH + "_set", "offset": 0}
    d = {"name": name, "engine": proto["engine"], "opcode": "DMACopy",
         "mode": "Copy", "cce_op": "bypass", "single_packet": False,
         "ins": [ap], "outs": [dict(ap, offset=2)],
         "sync_info": {"on_wait": [wait], "on_update": []}}
    for k in ("queue", "oob_is_err"):
        if k in proto:
            d[k] = proto[k]
    return d


def _split_multiwaits(bir_bytes):
    m = _orjson.loads(bir_bytes)
    for f in m.get("functions", []):
        for bb in f.get("blocks", []):
            out = []
            for ins in bb.get("instructions", []):
                si = ins.get("sync_info")
                waits = (si or {}).get("on_wait") or []
                if len(waits) > 1:
                    for k, w in enumerate(waits[:-1]):
                        nm = f"{ins['name']}-wsplit{k}"
                        out.append(_mk_nop(nm, ins["engine"], w))
                    si["on_wait"] = [waits[-1]]
                out.append(ins)
            bb["instructions"] = out
    return _orjson.dumps(m)


_BUILD_CACHE = {}


def _build_nc():
    nc = bass.Bass()
    x_in = nc.declare_dram_parameter("x", [5, 128, NROW, W], F16, isOutput=False)
    id_in = nc.declare_dram_parameter("ident", [128, 128], F16, isOutput=False)
    idw_in = nc.declare_dram_parameter("identw", [2, 128, 128], F16, isOutput=False)
    o_out = nc.declare_dram_parameter("out", [C, H, W], F16, isOutput=True)
    nc.dram_tensor(_SCRATCH, [4], F32)

    with tile.TileContext(nc) as tc:
        _emit(nc, tc, x_in, id_in, idw_in, o_out)

    orig_to_json = nc.to_json_bytes
    nc.to_json_bytes = lambda: _split_multiwaits(orig_to_json())
    return nc


def _emit(nc, tc, x_in, id_in, idw_in, o_out):
    from contextlib import ExitStack
    ctx = ExitStack()
    with ctx:
        persist = ctx.enter_context(tc.tile_pool(name="persist", bufs=1))
        p1 = ctx.enter_context(tc.tile_pool(name="p1", bufs=1))
        p2 = ctx.enter_context(tc.tile_pool(name="p2", bufs=2))
        p3 = ctx.enter_context(tc.tile_pool(name="p3", bufs=3))
        yz_p = ctx.enter_context(tc.tile_pool(name="yz", bufs=3))
        psum_p = ctx.enter_context(
            tc.tile_pool(name="psum", bufs=1, space=bass.MemorySpace.PSUM)
        )

        # ---- persistent fp16 input planes ----
        d16 = persist.tile([128, ALLOC], F16, tag="d16", name="d16")
        m16 = persist.tile([128, ALLOC], F16, tag="m16", name="m16")
        c16all = persist.tile([128, C, ALLOC], F16, tag="c16all", name="c16all")
        c16 = [c16all[:, i, :] for i in range(C)]
        ident = persist.tile([128, 128], F16, tag="ident", name="ident")
        identw = persist.tile([128, 2, 128], F16, tag="identw", name="identw")
        m3w0 = persist.tile([128, R, W], F16, tag="m3w0", name="m3w0")
        fhm = [persist.tile([128, EXT], F16, tag=f"fhm{i}", name=f"fhm{i}") for i in range(4)]
        ftm = [persist.tile([128, PROD], F16, tag=f"ftm{i}", name=f"ftm{i}") for i in range(4)]
        r32 = persist.tile([128, R, W], F32, tag="r32", name="r32")

        # zero only the pad regions (halo rows/cols arrive zeroed from the
        # host): cols 0-1 and 514-519 of each of the 6 rows, plus the slack.
        def v3_(ap_flat):
            return ap_flat[:, 0:FLAT].rearrange("p (a b) -> p a b", b=W2C)

        for t in (d16, m16, *c16):
            nc.vector.memset(v3_(t[:])[:, :, 0:COL0], 0.0)
            nc.vector.memset(v3_(t[:])[:, :, COL0 + W:W2C], 0.0)
            nc.vector.memset(t[:, FLAT:ALLOC], 0.0)

        # ---- load the 5 fp16 halo'd planes (built host-side) ----
        # x_in[k] is [128, 6, 512]: per-partition rows 4p-1..4p+4 with zero
        # halos, already fp16. One contiguous DMA per plane, alternating
        # between the two HWDGE rings; pads in the on-chip 520-wide layout
        # are zeroed once by the gpsimd memsets above.
        def v3(ap_flat):
            return ap_flat[:, 0:FLAT].rearrange("p (a b) -> p a b", b=W2C)

        planes = [d16, c16[0], c16[1], c16[2], m16]  # c16[i]: AP slice
        for k, dst16 in enumerate(planes):
            eng = nc.sync if k % 2 == 0 else nc.scalar
            eng.dma_start(
                v3(dst16[:])[:, 0:6, COL0:COL0 + W], x_in[k]
            )
        nc.scalar.dma_start(ident[:], id_in[:])
        nc.scalar.dma_start(identw[:], idw_in.rearrange("j p c -> p j c"))

        def oview(t16):  # [128,4,512] output-grid view of a fp16 plane
            return v3(t16[:])[:, 1:5, COL0:COL0 + W]

        PHI2 = 4.0 / math.pi  # (2/sqrt(pi))^2 from the two DErf factors
        nc.vector.tensor_scalar_mul(m3w0[:], oview(m16), 3.0 * W0 * PHI2)


        den = psum_p.tile([128, R, W], F32, tag="acc", name="den", bufs=2)

        # ---- phase A: per-pair fields + den accumulation ----
        # ws_e folds into the depth exp bias (g' = exp(-S t^2 + ln ws_e));
        # er==0 pairs only need field rows 1..4 (range starts at W2C).
        for i, (er, ec, ef, wse) in enumerate(ES):
            lo = W2C if er == 0 else 0
            td = p3.tile([128, EXT], F16, tag="tt", name="td")
            g = p2.tile([128, EXT], F16, tag="g", name="g")
            nc.vector.tensor_sub(
                td[:, lo:EXT], d16[:, lo + ef:EXT + ef], d16[:, lo:EXT]
            )
            nc.scalar.activation(
                g[:, lo:EXT], td[:, lo:EXT], AF.Derivative_Erf, scale=SQS
            )

            # packed color chain: one [3 x n] op per stage; square+exp run
            # in place on the ACT engine
            tca = p3.tile([128, C, EXT], F16, tag="tca", name="tca", bufs=2)
            nc.vector.tensor_sub(
                tca[:, :, lo:EXT], c16all[:, :, lo + ef:EXT + ef],
                c16all[:, :, lo:EXT]
            )
            nc.scalar.activation(
                tca[:, :, lo:EXT], tca[:, :, lo:EXT], AF.Derivative_Erf,
                scale=SQS,
            )

            G = p1.tile([128, EXT], F16, tag="G", name="G")
            nc.vector.tensor_add(
                G[:, lo:EXT], tca[:, 0, lo:EXT], tca[:, 1, lo:EXT]
            )
            nc.vector.tensor_add(G[:, lo:EXT], G[:, lo:EXT], tca[:, 2, lo:EXT])
            F = p1.tile([128, EXT], F16, tag="F", name="F")
            nc.vector.tensor_mul(F[:, lo:EXT], g[:, lo:EXT], G[:, lo:EXT])
            fl, fh = (W2C, EXT) if er == 0 else (0, PROD)
            nc.vector.tensor_mul(
                fhm[i][:, fl:fh], F[:, fl:fh], m16[:, fl:fh]
            )
            nc.vector.tensor_mul(
                ftm[i][:], F[:, W2C:EXT], m16[:, W2C + ef:EXT + ef]
            )
            for r in range(R):
                o = r * W2C + COL0
                nc.tensor.matmul(
                    den[:, r, :], identw[:, IWI[i], :], ftm[i][:, o:o + W],
                    start=(i == 0), stop=False,
                )
                ob = (1 + r) * W2C + COL0 - ef
                nc.tensor.matmul(
                    den[:, r, :], identw[:, IWI[i], :], fhm[i][:, ob:ob + W],
                    start=False, stop=False,
                )

        for r in range(R):
            nc.tensor.matmul(
                den[:, r, :], ident[:], m3w0[:, r, :],
                start=False, stop=(r == R - 1),
            )

        # 1/den = exp(-ln(den)); den>0 always (products of exps, positive mask).
        # Ln+Exp share one ACT table set; custom-DVE recip ops don't compile
        # with this walrus build, and InstReciprocal is ~8 cyc/elem.
        lden = persist.tile([128, R, W], F32, tag="lden", name="lden")
        nc.scalar.activation(lden[:], den[:], AF.Ln)
        nc.scalar.activation(r32[:], lden[:], AF.Exp, scale=-1.0)

        # ---- phase B: per-channel numerators ----
        r16 = persist.tile([128, R, W], F16, tag="r16", name="r16")
        nc.vector.tensor_copy(r16[:], r32[:])

        def _finals(num, ci):
            # num (PSUM fp32) -> SBUF fp16 via ACT, then a 2x fp16 multiply;
            # the fp16 output is upcast on the host.
            n16 = p1.tile([128, R, W], F16, tag="n16", name="n16", bufs=2)
            nc.scalar.activation(n16[:], num[:], AF.Copy)
            o16 = p1.tile([128, R, W], F16, tag="o16", name="o16", bufs=2)
            nc.vector.tensor_mul(o16[:], n16[:], r16[:])
            (nc.sync if ci % 2 == 0 else nc.scalar).dma_start(
                o_out[ci].rearrange("(p r) w -> p r w", r=R), o16[:]
            )

        pending = None
        for ci in range(C):
            num = psum_p.tile([128, R, W], F32, tag="acc", name="num", bufs=2)
            ncc = p1.tile([128, R, W], F16, tag="ncc", name="ncc")
            nc.vector.tensor_mul(ncc[:], m3w0[:], oview(c16[ci]))
            for i, (er, ec, ef, wse) in enumerate(ES):
                fl, fh = (W2C, EXT) if er == 0 else (0, PROD)
                Y = yz_p.tile([128, PROD], F16, tag="Y", name="Y")
                Z = yz_p.tile([128, EXT], F16, tag="Z", name="Z")
                nc.vector.tensor_mul(
                    Y[:], ftm[i][:], c16[ci][:, W2C + ef:EXT + ef]
                )
                nc.vector.tensor_mul(
                    Z[:, fl:fh], fhm[i][:, fl:fh], c16[ci][:, fl:fh]
                )
                for r in range(R):
                    o = r * W2C + COL0
                    nc.tensor.matmul(
                        num[:, r, :], identw[:, IWI[i], :], Y[:, o:o + W],
                        start=(i == 0), stop=False,
                    )
                    ob = (1 + r) * W2C + COL0 - ef
                    nc.tensor.matmul(
                        num[:, r, :], identw[:, IWI[i], :], Z[:, ob:ob + W],
                        start=False, stop=False,
                    )
            for r in range(R):
                nc.tensor.matmul(
                    num[:, r, :], ident[:], ncc[:, r, :],
                    start=False, stop=(r == R - 1),
                )
            if pending is not None:
                _finals(*pending)
            pending = (num, ci)
        _finals(*pending)


def _get_nc():
    if "nc" not in _BUILD_CACHE:
        _BUILD_CACHE["nc"] = _build_nc()
    return _BUILD_CACHE["nc"]


def _halo_planes(d, c, m):
    """[N,5,128,6,512] fp16: per-partition 6-row windows with zero halos."""
    from numpy.lib.stride_tricks import as_strided
    planes = np.concatenate(
        [d[:, None], np.moveaxis(c, 1, 1), m[:, None]], axis=1
    ) if False else None
    stack = np.empty((N_CORES, 5, H + 2, W), np.float16)
    for i in range(N_CORES):
        for k, arr in enumerate((d[i], c[i, 0], c[i, 1], c[i, 2], m[i])):
            stack[i, k, 1:H + 1] = arr
    stack[:, :, 0] = 0.0
    stack[:, :, H + 1] = 0.0
    # window p = padded rows 4p .. 4p+5  (padded row r == image row r-1)
    s = stack.strides
    win = as_strided(
        stack,
        shape=(N_CORES, 5, 128, NROW, W),
        strides=(s[0], s[1], 4 * s[2], s[2], s[3]),
    )
    return np.ascontiguousarray(win)


def _run(depth, color, mask, trace=False, **kw):
    nc = _get_nc()
    d = np.asarray(depth, dtype=np.float32).reshape(N_CORES, H, W)
    c = np.asarray(color, dtype=np.float32).reshape(N_CORES, C, H, W)
    m = np.asarray(mask, dtype=np.float32).reshape(N_CORES, H, W)
    x16 = _halo_planes(d, c, m)
    ident = np.eye(128, dtype=np.float16)
    identw = np.stack([np.eye(128) * W1, np.eye(128) * W2]).astype(np.float16)
    in_maps = [
        {"x": x16[i], "ident": ident, "identw": identw} for i in range(N_CORES)
    ]
    res = run_bass_kernel_spmd(
        nc, in_maps, list(range(N_CORES)), trace=trace, **kw
    )
    out = np.stack([np.asarray(res.results[i]["out"]) for i in range(N_CORES)])
    return out.reshape(B, V, C, H, W).astype(np.float32), res


def kernel(depth, color, mask):
    out, _ = _run(depth, color, mask, trace=False)
    return out
